# revision 13
# baseline (speedup 1.0000x reference)
"""Trainium2 Bass kernel for nn_CropCrossEntropy.

Reference computation (see reference.py):
    gt[i, y, x] = 1 inside the inclusive box [y0:y1, x0:x1] of image i, else 0
    loss = -(log(mp)*gt + log1p(-mp)*(1-gt)).mean()

Reformulation used here: with q = mp inside the box and q = 1-mp outside,
    loss = -mean(ln q),   q = sigma*(mp - 0.5) + 0.5,   sigma = 2*gt - 1.

sigma is a small-rank product of row/col box indicators, exactly
representable in bf16, so the TensorEngine builds it in PSUM from tiny
host-precomputed factors. Per element the device then does ONE VectorE op
u = (mp - 0.5) * sigma (scalar_tensor_tensor) and ONE ScalarE op
ln(2u + 1) = ln 2 + ln q (activation, free affine scale/bias, fused
per-partition accumulation); the host subtracts N*ln2. The kernel is
HBM-bandwidth bound (~16.8 MB/core).

Sharding: data-parallel over the fused (b*r)=512 image dim, 64 images/core
on 8 cores; each core returns per-partition partial sums; the host does the
final (tiny) reduction and the mean.

Per-core layout ("flat"): the 4 images of a chunk are one contiguous 1 MB
DRAM block viewed as [128, 2048] — partition p holds 2048 consecutive
floats = 8 consecutive rows of image (p//32). 8 KB contiguous DMA lines
per partition maximize DMA engine packet efficiency. For PSUM bank b
(columns [512b, 512b+512)), element (p, j') is image i=p//32, row
8*(p%32) + 2b + (j'//256), col j'%256 — so sigma for a bank is a K=9
matmul: rows (2i+h) pair [p//32==i]*rowind_i(8*(p%32)+2b+h) on the lhsT
side with 2*colind_i in column-half h on the rhs side, plus a constant
(ones x -1) row.
"""

from contextlib import ExitStack

import ml_dtypes
import numpy as np

import concourse.bass as bass
import concourse.tile as tile
from concourse import bacc, mybir
from concourse.bass_utils import run_bass_kernel_spmd

N_CORES = 8
B, R, H, W = 32, 16, 256, 256
IMGS = B * R                      # 512
IMGS_PER_CORE = IMGS // N_CORES   # 64
P = 128
CHUNK_IMGS = 4
N_CHUNKS = IMGS_PER_CORE // CHUNK_IMGS  # 16
CHUNK_FREE = CHUNK_IMGS * H * W // P    # 2048 (8 image rows per partition)
BANK = 512
N_BANKS = CHUNK_FREE // BANK      # 4
K = 9                             # mask rank: 4 images x 2 col-halves + const
N_ELEMS = IMGS * H * W
LN2 = float(np.log(2.0))

_cached_nc = None


def _build_nc():
    """Build + compile the (single-program SPMD) Bass kernel."""
    nc = bacc.Bacc("TRN2", target_bir_lowering=False, debug=False)

    mp = nc.dram_tensor(
        "mp", [N_CHUNKS * P, CHUNK_FREE], mybir.dt.float32, kind="ExternalInput"
    ).ap()
    # single combined mask tensor: [K, lhs (64*128) | rhs (16*512)]
    MASK_COLS = N_CHUNKS * N_BANKS * P + N_CHUNKS * BANK
    masks = nc.dram_tensor(
        "masks", [K, MASK_COLS], mybir.dt.bfloat16, kind="ExternalInput"
    ).ap()
    # last 2 chunks are processed in 2x1024 / 4x512 pieces -> 4 extra acc cols
    N_ACC = N_CHUNKS + 4
    acc_out = nc.dram_tensor(
        "acc", [P, N_ACC], mybir.dt.float32, kind="ExternalOutput"
    ).ap()

    with tile.TileContext(nc) as tc, ExitStack() as ctx:
        mask_pool = ctx.enter_context(tc.tile_pool(name="masks", bufs=1))
        mp_pool = ctx.enter_context(tc.tile_pool(name="mp", bufs=4))
        u_pool = ctx.enter_context(tc.tile_pool(name="u", bufs=3))
        scr_pool = ctx.enter_context(tc.tile_pool(name="scr", bufs=2))
        acc_pool = ctx.enter_context(tc.tile_pool(name="acc", bufs=1))
        ps_pool = ctx.enter_context(tc.tile_pool(name="sig", bufs=2, space="PSUM"))

        masks_t = mask_pool.tile([K, MASK_COLS], mybir.dt.bfloat16)
        # one DMA, issued before the mp chunks: the mask packets must land
        # before chunk 0 finishes or every matmul (and the whole pipeline)
        # stalls on them
        nc.sync.dma_start(masks_t[:], masks[:])
        mlhs_t = masks_t[:, : N_CHUNKS * N_BANKS * P]
        mrhs_t = masks_t[:, N_CHUNKS * N_BANKS * P :]

        acc_t = acc_pool.tile([P, N_ACC], mybir.dt.float32)

        acc_col = 0
        for c in range(N_CHUNKS):
            # taper the pipeline grain near the end: the last loads land as
            # smaller DMAs so DVE/ACT consume them as they arrive and the
            # post-DMA tail is one small piece, not a full 1MB chunk
            if c == N_CHUNKS - 2:
                n_pieces = 2
            elif c == N_CHUNKS - 1:
                n_pieces = 4
            else:
                n_pieces = 1
            piece = CHUNK_FREE // n_pieces

            mp_t = mp_pool.tile([P, CHUNK_FREE], mybir.dt.float32)
            for pc in range(n_pieces):
                lo, hi = pc * piece, (pc + 1) * piece
                nc.sync.dma_start(
                    mp_t[:, lo:hi], mp[c * P : (c + 1) * P, lo:hi]
                )

            # sigma = 2*gt - 1 in PSUM, one K=9 matmul per bank
            sg_t = ps_pool.tile([P, CHUNK_FREE], mybir.dt.float32)
            for b in range(N_BANKS):
                nc.tensor.matmul(
                    sg_t[:, b * BANK : (b + 1) * BANK],
                    mlhs_t[:, (c * N_BANKS + b) * P : (c * N_BANKS + b + 1) * P],
                    mrhs_t[:, c * BANK : (c + 1) * BANK],
                    start=True,
                    stop=True,
                )

            for pc in range(n_pieces):
                lo, hi = pc * piece, (pc + 1) * piece
                # u = (mp - 0.5) * sigma  (one DVE instruction)
                u_t = u_pool.tile([P, piece], mybir.dt.float32, tag="u")
                nc.vector.scalar_tensor_tensor(
                    u_t[:],
                    mp_t[:, lo:hi],
                    0.5,
                    sg_t[:, lo:hi],
                    mybir.AluOpType.subtract,
                    mybir.AluOpType.mult,
                )
                # ln(2u + 1) = ln2 + ln(q), fused per-partition accumulation
                scr_t = scr_pool.tile([P, piece], mybir.dt.float32, tag="scr")
                nc.scalar.activation(
                    scr_t[:],
                    u_t[:],
                    mybir.ActivationFunctionType.Ln,
                    bias=1.0,
                    scale=2.0,
                    accum_out=acc_t[:, acc_col : acc_col + 1],
                )
                acc_col += 1

            if c == N_CHUNKS - 3:
                # ship the bulk of acc early so only 6 columns remain at the end
                nc.sync.dma_start(
                    acc_out[:, : N_CHUNKS - 2], acc_t[:, : N_CHUNKS - 2]
                )

        nc.sync.dma_start(
            acc_out[:, N_CHUNKS - 2 :], acc_t[:, N_CHUNKS - 2 :]
        )

    nc.compile()
    return nc


def _get_nc():
    global _cached_nc
    if _cached_nc is None:
        _cached_nc = _build_nc()
    return _cached_nc


def _make_in_maps(mask_pred, pos_gt):
    mp = np.ascontiguousarray(np.asarray(mask_pred), dtype=np.float32).reshape(
        IMGS, H * W
    )
    pg = np.asarray(pos_gt).reshape(IMGS, 4).astype(np.int64)
    rows = np.arange(H)[None, :]
    cols = np.arange(W)[None, :]
    y0, x0, y1, x1 = (pg[:, k][:, None] for k in range(4))
    rowind = ((rows >= y0) & (rows <= y1)).astype(np.float32)  # (512, 256)
    colind = ((cols >= x0) & (cols <= x1)).astype(np.float32)  # (512, 256)

    # lhsT row for bank b: 8*(p%32) + 2b + h, p in [32i, 32i+32)
    q32 = np.arange(32)
    bank_rows = 8 * q32[None, :] + 2 * np.arange(N_BANKS)[:, None]  # (4, 32)

    in_maps = []
    for cid in range(N_CORES):
        sl = slice(cid * IMGS_PER_CORE, (cid + 1) * IMGS_PER_CORE)
        mp_c = mp[sl].reshape(N_CHUNKS * P, CHUNK_FREE)
        rc = rowind[sl].reshape(N_CHUNKS, CHUNK_IMGS, H)
        cc = colind[sl].reshape(N_CHUNKS, CHUNK_IMGS, W)

        lhs = np.zeros((N_CHUNKS, N_BANKS, K, P), np.float32)
        rhs = np.zeros((N_CHUNKS, K, BANK), np.float32)
        for i in range(CHUNK_IMGS):
            for h in range(2):
                # (chunks, banks, 32)
                lhs[:, :, 2 * i + h, 32 * i : 32 * (i + 1)] = rc[:, i][
                    :, bank_rows + h
                ]
                rhs[:, 2 * i + h, 256 * h : 256 * (h + 1)] = 2.0 * cc[:, i]
        lhs[:, :, 8, :] = 1.0
        rhs[:, 8, :] = -1.0

        mlhs = np.ascontiguousarray(
            lhs.reshape(N_CHUNKS * N_BANKS, K, P).transpose(1, 0, 2)
        ).reshape(K, -1)
        mrhs = np.ascontiguousarray(rhs.transpose(1, 0, 2)).reshape(K, -1)
        masks = np.concatenate([mlhs, mrhs], axis=1)
        in_maps.append(
            {
                "mp": mp_c,
                "masks": masks.astype(ml_dtypes.bfloat16),
            }
        )
    return in_maps


def _run(mask_pred, pos_gt, trace=False, **run_kwargs):
    nc = _get_nc()
    in_maps = _make_in_maps(mask_pred, pos_gt)
    res = run_bass_kernel_spmd(
        nc, in_maps, core_ids=list(range(N_CORES)), trace=trace, **run_kwargs
    )
    total = 0.0
    for r in res.results:
        total += float(np.sum(np.asarray(r["acc"], dtype=np.float64)))
    # acc sums ln(2u+1) = ln2 + ln(q): subtract the known N*ln2 shift
    loss = np.float32(-((total - N_ELEMS * LN2) / N_ELEMS))
    return loss, res


def kernel(mask_pred, pos_gt):
    loss, _ = _run(mask_pred, pos_gt, trace=False)
    return loss
